# revision 9
# baseline (speedup 1.0000x reference)
"""GAT (2-layer dense-graph attention over 4096 nodes) as a Trainium2
Bass/Tile SPMD kernel across 8 NeuronCores.

Sharding: attention destination rows are sharded 512/core for both layers.
Each core computes the full source-side quantities (h', d — tiny) from the
full x, and the s-scores only for its own 512 destination rows. An AllGather
exchanges the layer-0 output (transposed [65, 512] block per core, including
a ones-row used for bias folding) between the two layers.

Math notes (exactness): softmax_j(leakyrelu(s_i+d_j)) is invariant to any
per-row factor, so with E = exp(leakyrelu(z)) = max(e^z, e^{0.2 z}) we use
E' = E * e^{-0.2 s_i} = max(e^{0.8 s_i} e^{d_j}, e^{0.2 d_j}),
computed as ONE fused DVE tensor_scalar op per [128, 512] tile:
(a_tile * b_j) max c_j, with a = e^{0.8 s} replicated across partitions and
b = e^d, c = e^{0.2 d} as per-partition scalars. BatchNorm (eval mode) is
folded into the weights host-side.

Precision/perf: E is bf16 (single-pass PE matmuls instead of the fp32
LOW_HIGH double-pass; bf16 quantization of E largely cancels between the
softmax numerator and denominator). The aggregation values h' are kept at
~fp32 precision by splitting into bf16 high + bf16 residual parts placed at
partition-aligned stationary columns (0/32) with the softmax-denominator
ones-column at 64 — matmul cost is N-bound, so the extra columns are free.
Compute engines can only address partition bases 0/32/64/96, which dictates
those offsets; partition-shifted row assembly goes through sbuf->sbuf DMA.
"""

import numpy as np
import ml_dtypes

import concourse.bacc as bacc
import concourse.mybir as mybir
import concourse.tile as tile
from concourse import masks
from concourse.bass_utils import run_bass_kernel_spmd

F32 = mybir.dt.float32
BF16 = mybir.dt.float32  # bisect: fp32
N = 4096
NCORES = 8
RPC = N // NCORES          # destination rows per core = 512
NJT = N // 128             # 32 j-tiles of 128 source rows
BN_EPS = 1e-5

_CACHE = {}


def _build():
    nc = bacc.Bacc("TRN2", target_bir_lowering=False, debug=False,
                   num_devices=NCORES)

    x_d = nc.dram_tensor("x", [N, 32], F32, kind="ExternalInput")
    xs_d = nc.dram_tensor("x_slice", [RPC, 32], F32, kind="ExternalInput")
    w0all_d = nc.dram_tensor("w0all", [33, 80], F32, kind="ExternalInput")
    w0s_d = nc.dram_tensor("w0s", [33, 8], F32, kind="ExternalInput")
    w1all_d = nc.dram_tensor("w1all", [65, 33], F32, kind="ExternalInput")
    w1s_d = nc.dram_tensor("w1s", [65, 1], F32, kind="ExternalInput")
    b0_d = nc.dram_tensor("b0f", [64, 1], F32, kind="ExternalInput")
    b1_d = nc.dram_tensor("b1f", [32, 1], F32, kind="ExternalInput")
    sela_d = nc.dram_tensor("sela", [8, 8 * 128], F32, kind="ExternalInput")
    s0sel_d = nc.dram_tensor("s0sel", [8, 64], F32, kind="ExternalInput")
    out_d = nc.dram_tensor("out", [RPC, 32], F32, kind="ExternalOutput")

    with tile.TileContext(nc) as tc:
        with (
            tc.tile_pool(name="const", bufs=1) as const,
            tc.tile_pool(name="persist", bufs=1) as per,
            tc.tile_pool(name="dram", bufs=1, space="DRAM") as dram,
        ):
            ident = const.tile([128, 128], F32)
            masks.make_identity(nc, ident[:])
            ones_row = const.tile([1, 128], F32)
            nc.vector.memset(ones_row[:], 1.0)
            ones_row_bf = const.tile([1, 128], BF16)
            nc.vector.memset(ones_row_bf[:], 1.0)
            sela = const.tile([8, 8 * 128], F32)
            nc.sync.dma_start(sela[:], sela_d[:])
            s0sel = const.tile([8, 64], F32)
            nc.sync.dma_start(s0sel[:], s0sel_d[:])

            w0all = const.tile([33, 80], F32)
            nc.sync.dma_start(w0all[:], w0all_d[:])
            w0s = const.tile([33, 8], F32)
            nc.sync.dma_start(w0s[:], w0s_d[:])
            w1all = const.tile([65, 33], F32)
            nc.sync.dma_start(w1all[:], w1all_d[:])
            w1s = const.tile([65, 1], F32)
            nc.sync.dma_start(w1s[:], w1s_d[:])
            b0c = const.tile([64, 1], F32)
            nc.sync.dma_start(b0c[:], b0_d[:])
            b1c = const.tile([32, 1], F32)
            nc.sync.dma_start(b1c[:], b1_d[:])

            # big persistent sbuf tensors
            xT = per.tile([33, N], F32)        # x^T plus ones row
            xsT = per.tile([33, RPC], F32)     # x_slice^T plus ones row
            # stationary operand per (jt, h): hi(0:8) res(32:40) ones(64)
            hpa0 = per.tile([128, NJT, 8, 66], BF16)
            d0e = per.tile([128, NJT, 8], F32)       # e^{d0}
            d0e2 = per.tile([128, NJT, 8], F32)      # e^{0.2 d0}
            atile = per.tile([128, 8, 512], BF16)    # e^{0.8 s0} bcast
            outTN = per.tile([64, 512], F32)         # L0 numerators^T
            rows = per.tile([8, 512], F32)           # L0 denominators
            cont = per.tile([65, 512], F32)          # elu(out0)^T + ones row
            hTag = per.tile([65, 8, 512], F32)       # gathered h^T blocks
            # stationary per jt: hi(0:32) res(32:64) ones(64)
            hpa1 = per.tile([128, NJT, 66], BF16)
            d1e = per.tile([128, NJT], F32)
            d1e2 = per.tile([128, NJT], F32)
            a1tile = per.tile([128, 512], BF16)
            a0row = per.tile([8, 512], BF16)
            a1row = per.tile([1, 512], BF16)
            rrow = per.tile([8, 512], F32)
            rscr = per.tile([8, 512], F32)
            r1row = per.tile([1, 512], F32)
            r1scr = per.tile([1, 512], F32)
            num1 = per.tile([32, 512], F32)
            res1s = per.tile([32, 512], F32)
            norm1 = per.tile([32, 512], F32)

            cont_d = dram.tile([65, 512], F32)
            ag_d = dram.tile([NCORES * 65, 512], F32)

            # ---------------- Phase A: projections -----------------
            with (
                tc.tile_pool(name="ld", bufs=2) as ld,
                tc.tile_pool(name="tp", bufs=2, space="PSUM") as tp,
                tc.tile_pool(name="mm80", bufs=2, space="PSUM") as mm80,
                tc.tile_pool(name="pssa", bufs=2, space="PSUM") as pssa,
            ):
                # x -> xT (32 transposes), x_slice -> xsT (4 transposes)
                xbig = ld.tile([128, NJT, 32], F32, tag="xbig")
                nc.sync.dma_start(
                    xbig[:], x_d[:].rearrange("(k p) c -> p k c", p=128))
                for k in range(NJT):
                    pt = tp.tile([32, 128], F32)
                    nc.tensor.matmul(pt[:], xbig[:, k, :], ident[:, :],
                                     is_transpose=True)
                    nc.vector.tensor_copy(xT[0:32, k * 128:(k + 1) * 128],
                                          pt[:])
                nc.vector.memset(xT[32:33, :], 1.0)

                xsbig = ld.tile([128, 4, 32], F32, tag="xsbig")
                nc.sync.dma_start(
                    xsbig[:], xs_d[:].rearrange("(k p) c -> p k c", p=128))
                for k in range(4):
                    pt = tp.tile([32, 128], F32)
                    nc.tensor.matmul(pt[:], xsbig[:, k, :], ident[:, :],
                                     is_transpose=True)
                    nc.vector.tensor_copy(xsT[0:32, k * 128:(k + 1) * 128],
                                          pt[:])
                nc.vector.memset(xsT[32:33, :], 1.0)

                # h'0 (hi+res), d0 exps per j-tile
                nc.vector.memset(hpa0[:], 0.0)
                nc.vector.memset(hpa0[:, :, :, 64:65], 1.0)
                for jt in range(NJT):
                    p80 = mm80.tile([128, 80], F32)
                    nc.tensor.matmul(p80[:], xT[:, jt * 128:(jt + 1) * 128],
                                     w0all[:])
                    hsrc = p80[:, 0:64].rearrange("p (h o) -> p h o", h=8)
                    nc.vector.tensor_copy(hpa0[:, jt, :, 0:8], hsrc)
                    # residual = fp32 h' - bf16(h')
                    nc.vector.tensor_tensor(hpa0[:, jt, :, 32:40], hsrc,
                                            hpa0[:, jt, :, 0:8],
                                            op=mybir.AluOpType.subtract)
                    nc.scalar.activation(d0e[:, jt, :], p80[:, 64:72],
                                         mybir.ActivationFunctionType.Exp)
                    nc.scalar.activation(d0e2[:, jt, :], p80[:, 64:72],
                                         mybir.ActivationFunctionType.Exp,
                                         scale=0.2)

                # s0 rows for this core's 512 dst rows; a = e^{0.8 s}
                ps0 = pssa.tile([8, 512], F32, tag="ps0")
                nc.tensor.matmul(ps0[:], w0s[:], xsT[:])
                nc.scalar.activation(a0row[:], ps0[:],
                                     mybir.ActivationFunctionType.Exp,
                                     scale=0.8)
                for h in range(8):
                    pa = pssa.tile([128, 512], F32, tag="pa")
                    nc.tensor.matmul(pa[:], sela[:, h * 128:(h + 1) * 128],
                                     a0row[:])
                    nc.vector.tensor_copy(atile[:, h, :], pa[:])

            # ---------------- Phase B/C: layer-0 attention ----------------
            with (
                tc.tile_pool(name="epool", bufs=3) as epool,
                tc.tile_pool(name="agg", bufs=2, space="PSUM") as agg,
                tc.tile_pool(name="rb", bufs=1, space="PSUM") as rb,
                tc.tile_pool(name="tmp", bufs=1) as tmp,
            ):
                for h in range(8):
                    pg = agg.tile([65, 512], F32)
                    for jt in range(NJT):
                        e = epool.tile([128, 512], BF16, tag="e")
                        nc.vector.tensor_scalar(
                            e[:], atile[:, h, :],
                            d0e[:, jt, h:h + 1], d0e2[:, jt, h:h + 1],
                            op0=mybir.AluOpType.mult,
                            op1=mybir.AluOpType.max)
                        nc.tensor.matmul(pg[:], hpa0[:, jt, h, 0:65], e[:],
                                         start=(jt == 0), stop=(jt == NJT - 1))
                    # hi + residual numerators; engines address base 0/32/64
                    stgr = tmp.tile([8, 512], F32, tag="stgr")
                    nc.vector.tensor_copy(stgr[:], pg[32:40, :])
                    stgn = tmp.tile([8, 512], F32, tag="stgn")
                    nc.vector.tensor_tensor(stgn[:], pg[0:8, :], stgr[:],
                                            op=mybir.AluOpType.add)
                    stgd = tmp.tile([1, 512], F32, tag="stgd")
                    nc.vector.tensor_copy(stgd[:], pg[64:65, :])
                    nc.sync.dma_start(outTN[h * 8:(h + 1) * 8, :], stgn[:])
                    nc.sync.dma_start(rows[h:h + 1, :], stgd[:])

                # normalize + bias + ELU, build contribution [65, 512]
                nc.vector.reciprocal(rrow[:], rows[:])
                prb = rb.tile([64, 512], F32)
                nc.tensor.matmul(prb[:], s0sel[:], rrow[:])
                nrm = tmp.tile([64, 512], F32, tag="nrm")
                nc.vector.tensor_tensor(nrm[:], outTN[:], prb[:],
                                        op=mybir.AluOpType.mult)
                nc.vector.tensor_scalar_add(nrm[:], nrm[:], b0c[:])
                mneg = tmp.tile([64, 512], F32, tag="mneg")
                nc.vector.tensor_scalar_min(mneg[:], nrm[:], 0.0)
                eneg = tmp.tile([64, 512], F32, tag="eneg")
                nc.scalar.activation(eneg[:], mneg[:],
                                     mybir.ActivationFunctionType.Exp)
                ppos = tmp.tile([64, 512], F32, tag="ppos")
                nc.vector.tensor_scalar_max(ppos[:], nrm[:], 0.0)
                # cont = (eneg - 1) + ppos  == elu
                nc.vector.scalar_tensor_tensor(
                    cont[0:64, :], eneg[:], -1.0, ppos[:],
                    op0=mybir.AluOpType.add, op1=mybir.AluOpType.add)
                nc.vector.memset(cont[64:65, :], 1.0)

                nc.sync.dma_start(cont_d[:], cont[:])
                nc.gpsimd.collective_compute(
                    "AllGather",
                    mybir.AluOpType.bypass,
                    replica_groups=[list(range(NCORES))],
                    ins=[cont_d.opt()],
                    outs=[ag_d.opt()],
                )
                for blk in range(NCORES):
                    nc.sync.dma_start(hTag[:, blk, :],
                                      ag_d[blk * 65:(blk + 1) * 65, :])

            # ---------------- Phase D: layer 1 ----------------
            with (
                tc.tile_pool(name="e1pool", bufs=3) as e1pool,
                tc.tile_pool(name="mmd", bufs=2, space="PSUM") as mmd,
                tc.tile_pool(name="pd", bufs=1, space="PSUM") as pd,
                tc.tile_pool(name="agg1", bufs=1, space="PSUM") as agg1,
                tc.tile_pool(name="tp2", bufs=2, space="PSUM") as tp2,
                tc.tile_pool(name="ot", bufs=2) as ot,
            ):
                nc.vector.memset(hpa1[:, :, 64:65], 1.0)
                for jt in range(NJT):
                    blk, kk = jt // 4, jt % 4
                    p34 = mmd.tile([128, 33], F32)
                    nc.tensor.matmul(
                        p34[:], hTag[:, blk, kk * 128:(kk + 1) * 128],
                        w1all[:])
                    nc.vector.tensor_copy(hpa1[:, jt, 0:32], p34[:, 0:32])
                    nc.vector.tensor_tensor(hpa1[:, jt, 32:64], p34[:, 0:32],
                                            hpa1[:, jt, 0:32],
                                            op=mybir.AluOpType.subtract)
                    nc.scalar.activation(d1e[:, jt:jt + 1], p34[:, 32:33],
                                         mybir.ActivationFunctionType.Exp)
                    nc.scalar.activation(d1e2[:, jt:jt + 1], p34[:, 32:33],
                                         mybir.ActivationFunctionType.Exp,
                                         scale=0.2)

                ps1 = pd.tile([1, 512], F32, tag="ps1")
                nc.tensor.matmul(ps1[:], w1s[:], cont[:])
                nc.scalar.activation(a1row[:], ps1[:],
                                     mybir.ActivationFunctionType.Exp,
                                     scale=0.8)
                pa1 = pd.tile([128, 512], F32, tag="pa1")
                nc.tensor.matmul(pa1[:], ones_row_bf[:], a1row[:])
                nc.vector.tensor_copy(a1tile[:], pa1[:])

                pg1 = agg1.tile([65, 512], F32)
                for jt in range(NJT):
                    e1 = e1pool.tile([128, 512], BF16, tag="e1")
                    nc.vector.tensor_scalar(
                        e1[:], a1tile[:],
                        d1e[:, jt:jt + 1], d1e2[:, jt:jt + 1],
                        op0=mybir.AluOpType.mult,
                        op1=mybir.AluOpType.max)
                    nc.tensor.matmul(pg1[:], hpa1[:, jt, 0:65], e1[:],
                                     start=(jt == 0), stop=(jt == NJT - 1))

                nc.vector.reciprocal(r1row[:], pg1[64:65, :])
                prb1 = pd.tile([32, 512], F32, tag="prb1")
                nc.tensor.matmul(prb1[:], ones_row[0:1, 0:32], r1row[:])
                nc.vector.tensor_copy(res1s[:], pg1[32:64, :])
                nc.vector.tensor_tensor(num1[:], pg1[0:32, :], res1s[:],
                                        op=mybir.AluOpType.add)
                nc.vector.tensor_tensor(norm1[:], num1[:], prb1[:],
                                        op=mybir.AluOpType.mult)
                nc.vector.tensor_scalar_add(norm1[:], norm1[:], b1c[:])

                for ic in range(4):
                    pt2 = tp2.tile([128, 32], F32)
                    nc.tensor.matmul(pt2[:],
                                     norm1[:, ic * 128:(ic + 1) * 128],
                                     ident[0:32, 0:32], is_transpose=True)
                    ob = ot.tile([128, 32], F32, tag="ob")
                    nc.vector.tensor_copy(ob[:], pt2[:])
                    nc.sync.dma_start(out_d[ic * 128:(ic + 1) * 128, :],
                                      ob[:])

    nc.compile()
    return nc


def _fold(inputs):
    """Host-side BN folding and attention-projection folding (numpy)."""
    f64 = np.float64
    x = np.ascontiguousarray(np.asarray(inputs["x"], np.float32))
    w0 = np.asarray(inputs["w0"], f64)          # [8, 32, 8]
    w1 = np.asarray(inputs["w1"], f64)          # [1, 64, 32]
    a_src0 = np.asarray(inputs["a_src0"], f64)[..., 0]   # [8, 8]
    a_dst0 = np.asarray(inputs["a_dst0"], f64)[..., 0]   # [8, 8]
    a_src1 = np.asarray(inputs["a_src1"], f64)[0, :, 0]  # [32]
    a_dst1 = np.asarray(inputs["a_dst1"], f64)[0, :, 0]  # [32]
    b0 = np.asarray(inputs["b0"], f64)          # [8]
    b1 = np.asarray(inputs["b1"], f64)          # [32]

    al0 = np.asarray(inputs["bn0_gamma"], f64) / np.sqrt(
        np.asarray(inputs["bn0_var"], f64) + BN_EPS)
    sh0 = np.asarray(inputs["bn0_beta"], f64) - \
        np.asarray(inputs["bn0_mean"], f64) * al0
    al1 = np.asarray(inputs["bn1_gamma"], f64) / np.sqrt(
        np.asarray(inputs["bn1_var"], f64) + BN_EPS)
    sh1 = np.asarray(inputs["bn1_beta"], f64) - \
        np.asarray(inputs["bn1_mean"], f64) * al1

    # layer 0 folds
    w0flat = (al0[None, :, None] * w0).transpose(1, 0, 2).reshape(32, 64)
    beta0h = np.einsum("i,hio->ho", sh0, w0)     # [8, 8]
    beta0 = beta0h.reshape(64)
    as0 = al0[:, None] * np.einsum("hio,ho->ih", w0, a_src0)   # [32, 8]
    sb0 = np.einsum("ho,ho->h", beta0h, a_src0)
    ad0 = al0[:, None] * np.einsum("hio,ho->ih", w0, a_dst0)
    db0 = np.einsum("ho,ho->h", beta0h, a_dst0)

    w0all = np.zeros((33, 80), f64)
    w0all[0:32, 0:64] = w0flat
    w0all[32, 0:64] = beta0
    w0all[0:32, 64:72] = ad0
    w0all[32, 64:72] = db0
    w0s = np.zeros((33, 8), f64)
    w0s[0:32, :] = as0
    w0s[32, :] = sb0

    # layer 1 folds
    w1m = w1[0]                                   # [64, 32]
    w1flat = al1[:, None] * w1m
    beta1 = sh1 @ w1m                             # [32]
    as1 = al1 * (w1m @ a_src1)
    sb1 = beta1 @ a_src1
    ad1 = al1 * (w1m @ a_dst1)
    db1 = beta1 @ a_dst1

    w1all = np.zeros((65, 33), f64)
    w1all[0:64, 0:32] = w1flat
    w1all[64, 0:32] = beta1
    w1all[0:64, 32] = ad1
    w1all[64, 32] = db1
    w1s = np.zeros((65, 1), f64)
    w1s[0:64, 0] = as1
    w1s[64, 0] = sb1

    b0f = np.tile(b0, 8).reshape(64, 1)           # (h,o) flat -> b0[o]
    b1f = b1.reshape(32, 1)

    sela = np.zeros((8, 8, 128), np.float32)  # row h ones in block h
    for h in range(8):
        sela[h, h, :] = 1.0
    s0sel = np.zeros((8, 64), np.float32)         # S[h, m] = (m//8 == h)
    for h in range(8):
        s0sel[h, h * 8:(h + 1) * 8] = 1.0

    return {
        "x": x,
        "w0all": w0all.astype(np.float32),
        "w0s": w0s.astype(np.float32),
        "w1all": w1all.astype(np.float32),
        "w1s": w1s.astype(np.float32),
        "b0f": b0f.astype(np.float32),
        "b1f": b1f.astype(np.float32),
        "sela": sela.reshape(8, 8 * 128),
        "s0sel": s0sel,
    }


def kernel(**inputs) -> np.ndarray:
    if "nc" not in _CACHE:
        _CACHE["nc"] = _build()
    nc = _CACHE["nc"]

    shared = _fold(inputs)
    x = shared["x"]
    in_maps = []
    for c in range(NCORES):
        m = dict(shared)
        m["x_slice"] = np.ascontiguousarray(x[c * RPC:(c + 1) * RPC])
        in_maps.append(m)

    res = run_bass_kernel_spmd(nc, in_maps, list(range(NCORES)))
    out = np.concatenate([res.results[c]["out"] for c in range(NCORES)],
                         axis=0)
    return out.astype(np.float32)


# revision 10
# speedup vs baseline: 1.5816x; 1.5816x over previous
"""GAT (2-layer dense-graph attention over 4096 nodes) as a Trainium2
Bass/Tile SPMD kernel across 8 NeuronCores.

Sharding: attention destination rows are sharded 512/core for both layers.
Each core computes the full source-side quantities (h', d — tiny) from the
full x, and the s-scores only for its own 512 destination rows. An AllGather
exchanges the layer-0 output (transposed [65, 512] block per core, including
a ones-row used for bias folding) between the two layers.

Math notes (exactness): softmax_j(leakyrelu(s_i+d_j)) is invariant to any
per-row factor, so with E = exp(leakyrelu(z)) = max(e^z, e^{0.2 z}) we use
E' = E * e^{-0.2 s_i} = max(e^{0.8 s_i} e^{d_j}, e^{0.2 d_j}),
computed as ONE fused DVE tensor_scalar op per [128, 512] tile:
(a_tile * b_j) max c_j, with a = e^{0.8 s} replicated across partitions and
b = e^d, c = e^{0.2 d} as per-partition scalars. BatchNorm (eval mode) is
folded into the weights host-side.

Precision/perf: E is bf16 (single-pass PE matmuls instead of the fp32
LOW_HIGH double-pass; bf16 quantization of E largely cancels between the
softmax numerator and denominator). The aggregation values h' are kept at
~fp32 precision by splitting into bf16 high + bf16 residual parts placed at
partition-aligned stationary columns (0/32) with the softmax-denominator
ones-column at 64 — matmul cost is N-bound, so the extra columns are free.
Compute engines can only address partition bases 0/32/64/96, which dictates
those offsets; partition-shifted row assembly goes through sbuf->sbuf DMA.
"""

import numpy as np
import ml_dtypes

import concourse.bacc as bacc
import concourse.mybir as mybir
import concourse.tile as tile
from concourse import masks
from concourse.bass_utils import run_bass_kernel_spmd

F32 = mybir.dt.float32
BF16 = mybir.dt.bfloat16
N = 4096
NCORES = 8
RPC = N // NCORES          # destination rows per core = 512
NJT = N // 128             # 32 j-tiles of 128 source rows
BN_EPS = 1e-5

_CACHE = {}


def _build():
    nc = bacc.Bacc("TRN2", target_bir_lowering=False, debug=False,
                   num_devices=NCORES)

    x_d = nc.dram_tensor("x", [N, 32], F32, kind="ExternalInput")
    xs_d = nc.dram_tensor("x_slice", [RPC, 32], F32, kind="ExternalInput")
    w0all_d = nc.dram_tensor("w0all", [33, 80], F32, kind="ExternalInput")
    w0s_d = nc.dram_tensor("w0s", [33, 8], F32, kind="ExternalInput")
    w1all_d = nc.dram_tensor("w1all", [65, 33], F32, kind="ExternalInput")
    w1s_d = nc.dram_tensor("w1s", [65, 1], F32, kind="ExternalInput")
    b0_d = nc.dram_tensor("b0f", [64, 1], F32, kind="ExternalInput")
    b1_d = nc.dram_tensor("b1f", [32, 1], F32, kind="ExternalInput")
    sela_d = nc.dram_tensor("sela", [8, 8 * 128], BF16, kind="ExternalInput")
    s0sel_d = nc.dram_tensor("s0sel", [8, 64], F32, kind="ExternalInput")
    out_d = nc.dram_tensor("out", [RPC, 32], F32, kind="ExternalOutput")

    with tile.TileContext(nc) as tc:
        with (
            tc.tile_pool(name="const", bufs=1) as const,
            tc.tile_pool(name="persist", bufs=1) as per,
            tc.tile_pool(name="dram", bufs=1, space="DRAM") as dram,
        ):
            ident = const.tile([128, 128], F32)
            masks.make_identity(nc, ident[:])
            ones_row = const.tile([1, 128], F32)
            nc.vector.memset(ones_row[:], 1.0)
            ones_row_bf = const.tile([1, 128], BF16)
            nc.vector.memset(ones_row_bf[:], 1.0)
            sela = const.tile([8, 8 * 128], BF16)
            nc.sync.dma_start(sela[:], sela_d[:])
            s0sel = const.tile([8, 64], F32)
            nc.sync.dma_start(s0sel[:], s0sel_d[:])

            w0all = const.tile([33, 80], F32)
            nc.sync.dma_start(w0all[:], w0all_d[:])
            w0s = const.tile([33, 8], F32)
            nc.sync.dma_start(w0s[:], w0s_d[:])
            w1all = const.tile([65, 33], F32)
            nc.sync.dma_start(w1all[:], w1all_d[:])
            w1s = const.tile([65, 1], F32)
            nc.sync.dma_start(w1s[:], w1s_d[:])
            b0c = const.tile([64, 1], F32)
            nc.sync.dma_start(b0c[:], b0_d[:])
            b1c = const.tile([32, 1], F32)
            nc.sync.dma_start(b1c[:], b1_d[:])

            # big persistent sbuf tensors
            xT = per.tile([33, N], F32)        # x^T plus ones row
            xsT = per.tile([33, RPC], F32)     # x_slice^T plus ones row
            # stationary operand per (jt, h): hi(0:8) res(32:40) ones(64)
            hpa0 = per.tile([128, NJT, 8, 66], BF16)
            d0e = per.tile([128, NJT, 8], F32)       # e^{d0}
            d0e2 = per.tile([128, NJT, 8], F32)      # e^{0.2 d0}
            atile = per.tile([128, 8, 512], BF16)    # e^{0.8 s0} bcast
            outTN = per.tile([64, 512], F32)         # L0 numerators^T
            rows = per.tile([8, 512], F32)           # L0 denominators
            cont = per.tile([65, 512], F32)          # elu(out0)^T + ones row
            hTag = per.tile([65, 8, 512], F32)       # gathered h^T blocks
            # stationary per jt: hi(0:32) res(32:64) ones(64)
            hpa1 = per.tile([128, NJT, 66], BF16)
            d1e = per.tile([128, NJT], F32)
            d1e2 = per.tile([128, NJT], F32)
            a1tile = per.tile([128, 512], BF16)
            a0row = per.tile([8, 512], BF16)
            a1row = per.tile([1, 512], BF16)
            rrow = per.tile([8, 512], F32)
            rscr = per.tile([8, 512], F32)
            r1row = per.tile([1, 512], F32)
            r1scr = per.tile([1, 512], F32)
            num1 = per.tile([32, 512], F32)
            res1s = per.tile([32, 512], F32)
            norm1 = per.tile([32, 512], F32)

            cont_d = dram.tile([65, 512], F32)
            ag_d = dram.tile([NCORES * 65, 512], F32)

            # ---------------- Phase A: projections -----------------
            with (
                tc.tile_pool(name="ld", bufs=2) as ld,
                tc.tile_pool(name="tp", bufs=2, space="PSUM") as tp,
                tc.tile_pool(name="mm80", bufs=2, space="PSUM") as mm80,
                tc.tile_pool(name="pssa", bufs=2, space="PSUM") as pssa,
            ):
                # x -> xT (32 transposes), x_slice -> xsT (4 transposes)
                xbig = ld.tile([128, NJT, 32], F32, tag="xbig")
                nc.sync.dma_start(
                    xbig[:], x_d[:].rearrange("(k p) c -> p k c", p=128))
                for k in range(NJT):
                    pt = tp.tile([32, 128], F32)
                    nc.tensor.matmul(pt[:], xbig[:, k, :], ident[:, :],
                                     is_transpose=True)
                    nc.vector.tensor_copy(xT[0:32, k * 128:(k + 1) * 128],
                                          pt[:])
                nc.vector.memset(xT[32:33, :], 1.0)

                xsbig = ld.tile([128, 4, 32], F32, tag="xsbig")
                nc.sync.dma_start(
                    xsbig[:], xs_d[:].rearrange("(k p) c -> p k c", p=128))
                for k in range(4):
                    pt = tp.tile([32, 128], F32)
                    nc.tensor.matmul(pt[:], xsbig[:, k, :], ident[:, :],
                                     is_transpose=True)
                    nc.vector.tensor_copy(xsT[0:32, k * 128:(k + 1) * 128],
                                          pt[:])
                nc.vector.memset(xsT[32:33, :], 1.0)

                # h'0 (hi+res), d0 exps per j-tile
                nc.vector.memset(hpa0[:], 0.0)
                nc.vector.memset(hpa0[:, :, :, 64:65], 1.0)
                for jt in range(NJT):
                    p80 = mm80.tile([128, 80], F32)
                    nc.tensor.matmul(p80[:], xT[:, jt * 128:(jt + 1) * 128],
                                     w0all[:])
                    hsrc = p80[:, 0:64].rearrange("p (h o) -> p h o", h=8)
                    nc.vector.tensor_copy(hpa0[:, jt, :, 0:8], hsrc)
                    # residual = fp32 h' - bf16(h')
                    nc.vector.tensor_tensor(hpa0[:, jt, :, 32:40], hsrc,
                                            hpa0[:, jt, :, 0:8],
                                            op=mybir.AluOpType.subtract)
                    nc.scalar.activation(d0e[:, jt, :], p80[:, 64:72],
                                         mybir.ActivationFunctionType.Exp)
                    nc.scalar.activation(d0e2[:, jt, :], p80[:, 64:72],
                                         mybir.ActivationFunctionType.Exp,
                                         scale=0.2)

                # s0 rows for this core's 512 dst rows; a = e^{0.8 s}
                ps0 = pssa.tile([8, 512], F32, tag="ps0")
                nc.tensor.matmul(ps0[:], w0s[:], xsT[:])
                nc.scalar.activation(a0row[:], ps0[:],
                                     mybir.ActivationFunctionType.Exp,
                                     scale=0.8)
                for h in range(8):
                    pa = pssa.tile([128, 512], F32, tag="pa")
                    nc.tensor.matmul(pa[:], sela[:, h * 128:(h + 1) * 128],
                                     a0row[:])
                    nc.vector.tensor_copy(atile[:, h, :], pa[:])

            # ---------------- Phase B/C: layer-0 attention ----------------
            with (
                tc.tile_pool(name="epool", bufs=3) as epool,
                tc.tile_pool(name="agg", bufs=2, space="PSUM") as agg,
                tc.tile_pool(name="rb", bufs=1, space="PSUM") as rb,
                tc.tile_pool(name="tmp", bufs=1) as tmp,
            ):
                for h in range(8):
                    pg = agg.tile([65, 512], F32)
                    for jt in range(NJT):
                        e = epool.tile([128, 512], BF16, tag="e")
                        nc.vector.tensor_scalar(
                            e[:], atile[:, h, :],
                            d0e[:, jt, h:h + 1], d0e2[:, jt, h:h + 1],
                            op0=mybir.AluOpType.mult,
                            op1=mybir.AluOpType.max)
                        nc.tensor.matmul(pg[:], hpa0[:, jt, h, 0:65], e[:],
                                         start=(jt == 0), stop=(jt == NJT - 1))
                    # hi + residual numerators; engines address base 0/32/64
                    stgr = tmp.tile([8, 512], F32, tag="stgr")
                    nc.vector.tensor_copy(stgr[:], pg[32:40, :])
                    stgn = tmp.tile([8, 512], F32, tag="stgn")
                    nc.vector.tensor_tensor(stgn[:], pg[0:8, :], stgr[:],
                                            op=mybir.AluOpType.add)
                    stgd = tmp.tile([1, 512], F32, tag="stgd")
                    nc.vector.tensor_copy(stgd[:], pg[64:65, :])
                    nc.sync.dma_start(outTN[h * 8:(h + 1) * 8, :], stgn[:])
                    nc.sync.dma_start(rows[h:h + 1, :], stgd[:])

                # normalize + bias + ELU, build contribution [65, 512]
                nc.vector.reciprocal(rrow[:], rows[:])
                prb = rb.tile([64, 512], F32)
                nc.tensor.matmul(prb[:], s0sel[:], rrow[:])
                nrm = tmp.tile([64, 512], F32, tag="nrm")
                nc.vector.tensor_tensor(nrm[:], outTN[:], prb[:],
                                        op=mybir.AluOpType.mult)
                nc.vector.tensor_scalar_add(nrm[:], nrm[:], b0c[:])
                mneg = tmp.tile([64, 512], F32, tag="mneg")
                nc.vector.tensor_scalar_min(mneg[:], nrm[:], 0.0)
                eneg = tmp.tile([64, 512], F32, tag="eneg")
                nc.scalar.activation(eneg[:], mneg[:],
                                     mybir.ActivationFunctionType.Exp)
                ppos = tmp.tile([64, 512], F32, tag="ppos")
                nc.vector.tensor_scalar_max(ppos[:], nrm[:], 0.0)
                # cont = (eneg - 1) + ppos  == elu
                nc.vector.scalar_tensor_tensor(
                    cont[0:64, :], eneg[:], -1.0, ppos[:],
                    op0=mybir.AluOpType.add, op1=mybir.AluOpType.add)
                nc.vector.memset(cont[64:65, :], 1.0)

                nc.sync.dma_start(cont_d[:], cont[:])
                nc.gpsimd.collective_compute(
                    "AllGather",
                    mybir.AluOpType.bypass,
                    replica_groups=[list(range(NCORES))],
                    ins=[cont_d.opt()],
                    outs=[ag_d.opt()],
                )
                for blk in range(NCORES):
                    nc.sync.dma_start(hTag[:, blk, :],
                                      ag_d[blk * 65:(blk + 1) * 65, :])

            # ---------------- Phase D: layer 1 ----------------
            with (
                tc.tile_pool(name="e1pool", bufs=3) as e1pool,
                tc.tile_pool(name="mmd", bufs=2, space="PSUM") as mmd,
                tc.tile_pool(name="pd", bufs=1, space="PSUM") as pd,
                tc.tile_pool(name="agg1", bufs=1, space="PSUM") as agg1,
                tc.tile_pool(name="tp2", bufs=2, space="PSUM") as tp2,
                tc.tile_pool(name="ot", bufs=2) as ot,
            ):
                nc.vector.memset(hpa1[:, :, 64:65], 1.0)
                for jt in range(NJT):
                    blk, kk = jt // 4, jt % 4
                    p34 = mmd.tile([128, 33], F32)
                    nc.tensor.matmul(
                        p34[:], hTag[:, blk, kk * 128:(kk + 1) * 128],
                        w1all[:])
                    nc.vector.tensor_copy(hpa1[:, jt, 0:32], p34[:, 0:32])
                    nc.vector.tensor_tensor(hpa1[:, jt, 32:64], p34[:, 0:32],
                                            hpa1[:, jt, 0:32],
                                            op=mybir.AluOpType.subtract)
                    nc.scalar.activation(d1e[:, jt:jt + 1], p34[:, 32:33],
                                         mybir.ActivationFunctionType.Exp)
                    nc.scalar.activation(d1e2[:, jt:jt + 1], p34[:, 32:33],
                                         mybir.ActivationFunctionType.Exp,
                                         scale=0.2)

                ps1 = pd.tile([1, 512], F32, tag="ps1")
                nc.tensor.matmul(ps1[:], w1s[:], cont[:])
                nc.scalar.activation(a1row[:], ps1[:],
                                     mybir.ActivationFunctionType.Exp,
                                     scale=0.8)
                pa1 = pd.tile([128, 512], F32, tag="pa1")
                nc.tensor.matmul(pa1[:], ones_row_bf[:], a1row[:])
                nc.vector.tensor_copy(a1tile[:], pa1[:])

                pg1 = agg1.tile([65, 512], F32)
                for jt in range(NJT):
                    e1 = e1pool.tile([128, 512], BF16, tag="e1")
                    nc.vector.tensor_scalar(
                        e1[:], a1tile[:],
                        d1e[:, jt:jt + 1], d1e2[:, jt:jt + 1],
                        op0=mybir.AluOpType.mult,
                        op1=mybir.AluOpType.max)
                    nc.tensor.matmul(pg1[:], hpa1[:, jt, 0:65], e1[:],
                                     start=(jt == 0), stop=(jt == NJT - 1))

                nc.vector.reciprocal(r1row[:], pg1[64:65, :])
                prb1 = pd.tile([32, 512], F32, tag="prb1")
                nc.tensor.matmul(prb1[:], ones_row[0:1, 0:32], r1row[:])
                nc.vector.tensor_copy(res1s[:], pg1[32:64, :])
                nc.vector.tensor_tensor(num1[:], pg1[0:32, :], res1s[:],
                                        op=mybir.AluOpType.add)
                nc.vector.tensor_tensor(norm1[:], num1[:], prb1[:],
                                        op=mybir.AluOpType.mult)
                nc.vector.tensor_scalar_add(norm1[:], norm1[:], b1c[:])

                for ic in range(4):
                    pt2 = tp2.tile([128, 32], F32)
                    nc.tensor.matmul(pt2[:],
                                     norm1[:, ic * 128:(ic + 1) * 128],
                                     ident[0:32, 0:32], is_transpose=True)
                    ob = ot.tile([128, 32], F32, tag="ob")
                    nc.vector.tensor_copy(ob[:], pt2[:])
                    nc.sync.dma_start(out_d[ic * 128:(ic + 1) * 128, :],
                                      ob[:])

    nc.compile()
    return nc


def _fold(inputs):
    """Host-side BN folding and attention-projection folding (numpy)."""
    f64 = np.float64
    x = np.ascontiguousarray(np.asarray(inputs["x"], np.float32))
    w0 = np.asarray(inputs["w0"], f64)          # [8, 32, 8]
    w1 = np.asarray(inputs["w1"], f64)          # [1, 64, 32]
    a_src0 = np.asarray(inputs["a_src0"], f64)[..., 0]   # [8, 8]
    a_dst0 = np.asarray(inputs["a_dst0"], f64)[..., 0]   # [8, 8]
    a_src1 = np.asarray(inputs["a_src1"], f64)[0, :, 0]  # [32]
    a_dst1 = np.asarray(inputs["a_dst1"], f64)[0, :, 0]  # [32]
    b0 = np.asarray(inputs["b0"], f64)          # [8]
    b1 = np.asarray(inputs["b1"], f64)          # [32]

    al0 = np.asarray(inputs["bn0_gamma"], f64) / np.sqrt(
        np.asarray(inputs["bn0_var"], f64) + BN_EPS)
    sh0 = np.asarray(inputs["bn0_beta"], f64) - \
        np.asarray(inputs["bn0_mean"], f64) * al0
    al1 = np.asarray(inputs["bn1_gamma"], f64) / np.sqrt(
        np.asarray(inputs["bn1_var"], f64) + BN_EPS)
    sh1 = np.asarray(inputs["bn1_beta"], f64) - \
        np.asarray(inputs["bn1_mean"], f64) * al1

    # layer 0 folds
    w0flat = (al0[None, :, None] * w0).transpose(1, 0, 2).reshape(32, 64)
    beta0h = np.einsum("i,hio->ho", sh0, w0)     # [8, 8]
    beta0 = beta0h.reshape(64)
    as0 = al0[:, None] * np.einsum("hio,ho->ih", w0, a_src0)   # [32, 8]
    sb0 = np.einsum("ho,ho->h", beta0h, a_src0)
    ad0 = al0[:, None] * np.einsum("hio,ho->ih", w0, a_dst0)
    db0 = np.einsum("ho,ho->h", beta0h, a_dst0)

    w0all = np.zeros((33, 80), f64)
    w0all[0:32, 0:64] = w0flat
    w0all[32, 0:64] = beta0
    w0all[0:32, 64:72] = ad0
    w0all[32, 64:72] = db0
    w0s = np.zeros((33, 8), f64)
    w0s[0:32, :] = as0
    w0s[32, :] = sb0

    # layer 1 folds
    w1m = w1[0]                                   # [64, 32]
    w1flat = al1[:, None] * w1m
    beta1 = sh1 @ w1m                             # [32]
    as1 = al1 * (w1m @ a_src1)
    sb1 = beta1 @ a_src1
    ad1 = al1 * (w1m @ a_dst1)
    db1 = beta1 @ a_dst1

    w1all = np.zeros((65, 33), f64)
    w1all[0:64, 0:32] = w1flat
    w1all[64, 0:32] = beta1
    w1all[0:64, 32] = ad1
    w1all[64, 32] = db1
    w1s = np.zeros((65, 1), f64)
    w1s[0:64, 0] = as1
    w1s[64, 0] = sb1

    b0f = np.tile(b0, 8).reshape(64, 1)           # (h,o) flat -> b0[o]
    b1f = b1.reshape(32, 1)

    sela = np.zeros((8, 8, 128), ml_dtypes.bfloat16)  # row h ones in block h
    for h in range(8):
        sela[h, h, :] = 1.0
    s0sel = np.zeros((8, 64), np.float32)         # S[h, m] = (m//8 == h)
    for h in range(8):
        s0sel[h, h * 8:(h + 1) * 8] = 1.0

    return {
        "x": x,
        "w0all": w0all.astype(np.float32),
        "w0s": w0s.astype(np.float32),
        "w1all": w1all.astype(np.float32),
        "w1s": w1s.astype(np.float32),
        "b0f": b0f.astype(np.float32),
        "b1f": b1f.astype(np.float32),
        "sela": sela.reshape(8, 8 * 128),
        "s0sel": s0sel,
    }


def kernel(**inputs) -> np.ndarray:
    if "nc" not in _CACHE:
        _CACHE["nc"] = _build()
    nc = _CACHE["nc"]

    shared = _fold(inputs)
    x = shared["x"]
    in_maps = []
    for c in range(NCORES):
        m = dict(shared)
        m["x_slice"] = np.ascontiguousarray(x[c * RPC:(c + 1) * RPC])
        in_maps.append(m)

    res = run_bass_kernel_spmd(nc, in_maps, list(range(NCORES)))
    out = np.concatenate([res.results[c]["out"] for c in range(NCORES)],
                         axis=0)
    return out.astype(np.float32)
